# revision 9
# baseline (speedup 1.0000x reference)
"""Bass/Tile TRN2 kernel for nn_CausalAttention (softmax + tril-matmul renorm).

Math restructuring (per core, row block of B = SEQ/n_cores rows):
    q = x @ wq ; k = x @ wk ; v = x @ wv
    z = q @ k.T / sqrt(D) ;  s = exp(z)              (softmax norm cancels below)
    masked[i,j] = sum_{t>=j} s[i,t]                  (suffix sum == s @ tril)
    out = (masked @ v) / rowsum(masked)

v2 identities (vs the tril/suffix formulation):
    masked @ v       == s @ cumsum(v)                 -> contract s with prefix-V
    rowsum(masked)   == s @ (t+1)                     -> one weight column
    z = q @ k.T      == ((x@wq) @ wk.T) @ x.T         -> gather RAW x, not K

so the AllGather input (x.T in fp8) is ready ~6us into the kernel instead of
after a full projection, and the O(S^2) tril matmuls + psum copies vanish.

Per-tile decomposition (tile r of 128 keys, T tiles):
    Vc[rP+j] = Vc0_r[j] + sum_{r'<r} VS[r']           (within-tile prefix + offsets)
    numT = sum_r Vc0_r.T-mm(st_r) + VS.T-mm(SUF)      SUF[r] = sum_{r'>r} CS[r']
    den[i] = sum_t (t+1) s[t,i]                       (selector-pair matmul w/ CS)

Layouts: everything transposed ([feature/key on partitions, query on free]).
Prefix-x trick: the x-transpose matmuls use rhs=[I | U] (U=upper-tri ones) so a
single pass yields both x.T (fp8, scores+gather) and prefix-x.T (bf16, V path);
xrs (tile row-sums of x) is the last U-column, read from PSUM in f32.

Collectives: cc1 = AllGather(x.T fp8) triggered right after the transposes;
cc2 = AllGather(Vc0 fp8 + xrs bf16) after the V projection. Both on the
otherwise-empty GpSimd queue so nothing delays the trigger.
"""
import numpy as np
from contextlib import ExitStack

import concourse.bass as bass
import concourse.tile as tile
from concourse import bacc, mybir

F32 = mybir.dt.float32
BF16 = mybir.dt.bfloat16
FP8 = mybir.dt.float8e4
U8 = mybir.dt.uint8
AX = mybir.AxisListType
AF = mybir.ActivationFunctionType
ALU = mybir.AluOpType

P = 128
MB = 48          # selector pair block half-width (padded for DoubleRow step%16)


def make_consts(T):
    iu = np.concatenate([np.eye(P, dtype=np.float32),
                         np.triu(np.ones((P, P), np.float32))], axis=1)
    stril = np.tri(T, T, -1, dtype=np.float32)  # [r', r] = 1 if r' > r
    # selector pair blocks (DoubleRow over tile pairs a=2pr, b=2pr+1):
    # ko0 col a / ko1 col b = 1 (per-tile colsum -> CS rows); col 32 = den
    # weight (t+1)/32, pinned to partition 32 for the later row extraction.
    NPAIR = T // 2
    selp = np.zeros((P, NPAIR * 2 * MB), np.float32)
    for pr in range(NPAIR):
        a, b = 2 * pr, 2 * pr + 1
        blk = pr * 2 * MB
        selp[:, blk + a] = 1.0
        selp[:, blk + 32] = (P * a + np.arange(P) + 1.0) / 32.0
        selp[:, blk + MB + b] = 1.0
        selp[:, blk + MB + 32] = (P * b + np.arange(P) + 1.0) / 32.0
    ident = np.eye(P, dtype=np.float32)
    import ml_dtypes
    bf = lambda a: a.astype(ml_dtypes.bfloat16)
    f8 = lambda a: a.astype(ml_dtypes.float8_e4m3)
    return dict(c_iu=bf(iu), c_stril=bf(stril), c_selp=f8(selp), c_ident=ident)


def build(SEQ=4096, D=1024, n_cores=8):
    T = SEQ // P           # global 128-key tiles
    TL = T // n_cores      # local tiles per core
    B = P * TL             # rows per core
    DC = D // P            # feature chunks
    W = min(512, D)        # moving free width for D-wide matmuls
    NH = D // W
    NPAIR = T // 2
    assert B <= 512 and T <= P and D % W == 0 and SEQ % (P * n_cores) == 0
    # wq prescaled x8, wk.T prescaled x2 -> z = 512 * z_true
    scale = float(1.0 / np.sqrt(D) / 16.0)
    EXPB = float(-np.log(16.0))   # st = s/16 keeps fp8e4 range safe

    nc = bacc.Bacc("TRN2", target_bir_lowering=False, debug=False, num_devices=n_cores)

    x = nc.dram_tensor("x", [B, D], BF16, kind="ExternalInput")
    wq_d = nc.dram_tensor("wq", [D, D], FP8, kind="ExternalInput")
    wkt_d = nc.dram_tensor("wk", [D, D], FP8, kind="ExternalInput")   # wk.T * 2
    wv_d = nc.dram_tensor("wv", [D, D], BF16, kind="ExternalInput")
    c_iu = nc.dram_tensor("c_iu", [P, 2 * P], BF16, kind="ExternalInput")
    c_stril = nc.dram_tensor("c_stril", [T, T], BF16, kind="ExternalInput")
    c_selp = nc.dram_tensor("c_selp", [P, NPAIR * 2 * MB], FP8, kind="ExternalInput")
    c_ident = nc.dram_tensor("c_ident", [P, P], F32, kind="ExternalInput")
    out = nc.dram_tensor("out", [B, D], F32, kind="ExternalOutput")

    KH = D * B             # cc1: xT8 region [P, DC*B] fp8, flat (p k)
    VNB = B * D            # cc2 bytes: Vc0 region [P, TL*D] fp8e4
    XRB = 2 * D * TL       # cc2 bytes: xrs region [P, DC*TL] bf16
    CC2N = VNB + XRB

    with tile.TileContext(nc) as tc, ExitStack() as top:
        dram = top.enter_context(tc.tile_pool(name="dram", bufs=1, space="DRAM"))
        cc1_in = dram.tile([KH], FP8)
        cc1_out = dram.tile([n_cores, KH], FP8, addr_space="Shared")
        cc2_in = dram.tile([CC2N], U8)
        cc2_out = dram.tile([n_cores, CC2N], U8, addr_space="Shared")

        consts = top.enter_context(tc.tile_pool(name="consts", bufs=1))
        iu_sb = consts.tile([P, 2 * P], BF16)
        nc.scalar.dma_start(iu_sb[:], c_iu.ap())
        stril_sb = consts.tile([T, T], BF16)
        nc.scalar.dma_start(stril_sb[:], c_stril.ap())
        selp_sb = consts.tile([P, NPAIR * 2 * MB], FP8)
        nc.scalar.dma_start(selp_sb[:], c_selp.ap())
        ident_sb = consts.tile([P, P], F32)
        nc.scalar.dma_start(ident_sb[:], c_ident.ap())
        expb_sb = consts.tile([P, 1], F32)
        nc.vector.memset(expb_sb[:], EXPB)

        persist = top.enter_context(tc.tile_pool(name="persist", bufs=1))
        q2T = persist.tile([P, DC * B], FP8)         # (q @ wk.T).T row block
        st = persist.tile([P, T * B], FP8)           # exp(scores)/16, transposed
        vs_sb = persist.tile([T, D], BF16)           # per-tile V colsums
        xrs_g = persist.tile([P, DC * T], BF16)      # gathered per-tile x row sums
        suf_sb = persist.tile([T, B], BF16)
        cs_sb = persist.tile([T, B], BF16)
        recip = persist.tile([P, TL], F32)
        dennat = persist.tile([P, TL], F32)
        den_pad = persist.tile([P, B], F32)
        wv_sb = persist.tile([P, DC * D], BF16)
        vps = [persist.tile([P, TL * D], FP8, name=f"vp_{rc}")
               for rc in range(n_cores)]

        # ------------- stage 1: transposes, gather-x, projections -------------
        with ExitStack() as s1:
            xpool = s1.enter_context(tc.tile_pool(name="xload", bufs=6))
            xTp = s1.enter_context(tc.tile_pool(name="xT", bufs=1))
            xT8 = xTp.tile([P, DC * B], FP8)     # x.T   (scores lhsT + cc1 input)
            xcT = xTp.tile([P, DC * B], BF16)    # prefix-x.T (V path)
            xrs_f = xTp.tile([P, DC * TL], F32)
            xrs_bf = xTp.tile([P, DC * TL], BF16)

            wpool = s1.enter_context(tc.tile_pool(name="w", bufs=1))
            wq_sb = wpool.tile([P, DC * D], FP8)
            wkt_sb = wpool.tile([P, DC * D], FP8)
            qT = wpool.tile([P, DC * B], FP8)

            trps = s1.enter_context(tc.tile_pool(name="trps", bufs=2, space="PSUM"))
            # x.T and prefix-x.T in one pass: out = x_chunk.T @ [I | U]
            xts = []
            for tcc in range(TL):
                xt_ = xpool.tile([P, D], BF16, tag="x", name=f"xt_{tcc}")
                (nc.sync if tcc < 2 else nc.scalar).dma_start(
                    xt_[:], x.ap()[tcc * P:(tcc + 1) * P, :])
                xts.append(xt_)
            cc1v = cc1_in[0:KH].rearrange("(p k) -> p k", p=P)
            for dc in range(DC):
                psC = trps.tile([P, TL * 2 * P], F32, tag="tr")
                for tcc in range(TL):
                    nc.tensor.matmul(psC[:, tcc * 2 * P:(tcc + 1) * 2 * P],
                                     xts[tcc][:, dc * P:(dc + 1) * P], iu_sb[:],
                                     start=True, stop=True)
                v3 = psC.rearrange("p (t two j) -> p t two j", two=2, j=P)
                eng0 = nc.vector.tensor_copy if dc % 2 == 0 else nc.scalar.copy
                eng1 = nc.scalar.copy if dc % 2 == 0 else nc.vector.tensor_copy
                eng0(xT8[:, dc * B:(dc + 1) * B].rearrange("p (t j) -> p t j", j=P),
                     v3[:, :, 0, :])
                eng1(xcT[:, dc * B:(dc + 1) * B].rearrange("p (t j) -> p t j", j=P),
                     v3[:, :, 1, :])
                nc.vector.tensor_copy(
                    xrs_f[:, dc * TL:(dc + 1) * TL]
                    .rearrange("p (t one) -> p t one", one=1),
                    v3[:, :, 1, P - 1:P])
                # stream x.T chunks to the collective input as they land
                nc.sync.dma_start(cc1v[:, dc * B:(dc + 1) * B],
                                  xT8[:, dc * B:(dc + 1) * B])

            # gather x.T immediately (GpSimd queue is otherwise empty)
            nc.gpsimd.collective_compute(
                "AllGather", ALU.bypass,
                replica_groups=[list(range(n_cores))],
                ins=[cc1_in.opt()], outs=[cc1_out.opt()],
            )

            # weights (HWDGE queues; nothing gates the collective).
            # wv first: the V path gates cc2, which gates phase B.
            for dc in range(DC):
                nc.scalar.dma_start(wv_sb[:, dc * D:(dc + 1) * D], wv_d.ap()[dc * P:(dc + 1) * P, :])
            for dc in range(DC):
                nc.sync.dma_start(wq_sb[:, dc * D:(dc + 1) * D], wq_d.ap()[dc * P:(dc + 1) * P, :])
            for dc in range(DC):
                nc.scalar.dma_start(wkt_sb[:, dc * D:(dc + 1) * D], wkt_d.ap()[dc * P:(dc + 1) * P, :])

            pps = s1.enter_context(tc.tile_pool(name="pps", bufs=4, space="PSUM"))

            # Vc0 row blocks (within-tile prefix V, from prefix-x.T) -> cc2.
            # Emitted before the q projections: cc2 rides right behind cc1.
            cc2v = cc2_in[0:VNB].rearrange("(p k) -> p k", p=P)
            for tcc in range(TL):
                vl = xpool.tile([P, D], FP8, tag="vl")
                for nh in range(NH):
                    v_ps = pps.tile([P, W], F32, tag="pp", name="v_ps")
                    for dci in range(DC):
                        nc.tensor.matmul(
                            v_ps[:],
                            xcT[:, dci * B + tcc * P: dci * B + (tcc + 1) * P],
                            wv_sb[:, dci * D + nh * W: dci * D + (nh + 1) * W],
                            start=(dci == 0), stop=(dci == DC - 1),
                        )
                    (nc.vector.tensor_copy if nh % 2 == 0 else nc.scalar.copy)(
                        vl[:, nh * W:(nh + 1) * W], v_ps[:])
                nc.gpsimd.dma_start(cc2v[:, tcc * D:(tcc + 1) * D], vl[:].bitcast(U8))

            # per-tile x row sums (f32 exact, from the U-part last column)
            nc.vector.tensor_copy(xrs_bf[:], xrs_f[:])
            cc2x = cc2_in[VNB:VNB + XRB].rearrange("(p k) -> p k", p=P)
            nc.gpsimd.dma_start(cc2x[:, :], xrs_bf[:].bitcast(U8))

            # qT = (x @ wq).T  then  q2T = (q @ wk.T).T   (both fp8 DoubleRow)
            wq3 = wq_sb.rearrange("p (dc d) -> p dc d", dc=DC)
            wkt3 = wkt_sb.rearrange("p (dc d) -> p dc d", dc=DC)
            xT83 = xT8.rearrange("p (dc b) -> p dc b", dc=DC)
            for dco in range(DC):
                q_ps = pps.tile([P, B], F32, tag="pp", name="q_ps")
                for pp_ in range(DC // 2):
                    nc.tensor.matmul(
                        q_ps[:],
                        wq3[:, 2 * pp_:2 * pp_ + 2, dco * P:(dco + 1) * P],
                        xT83[:, 2 * pp_:2 * pp_ + 2, :],
                        start=(pp_ == 0), stop=(pp_ == DC // 2 - 1),
                        perf_mode=mybir.MatmulPerfMode.DoubleRow,
                    )
                nc.vector.tensor_copy(qT[:, dco * B:(dco + 1) * B], q_ps[:])
            qT3 = qT.rearrange("p (dc b) -> p dc b", dc=DC)
            for dco in range(DC):
                q2_ps = pps.tile([P, B], F32, tag="pp", name="q2_ps")
                for pp_ in range(DC // 2):
                    nc.tensor.matmul(
                        q2_ps[:],
                        wkt3[:, 2 * pp_:2 * pp_ + 2, dco * P:(dco + 1) * P],
                        qT3[:, 2 * pp_:2 * pp_ + 2, :],
                        start=(pp_ == 0), stop=(pp_ == DC // 2 - 1),
                        perf_mode=mybir.MatmulPerfMode.DoubleRow,
                    )
                nc.vector.tensor_copy(q2T[:, dco * B:(dco + 1) * B], q2_ps[:])

        # second collective: gather Vc0 + xrs
        nc.gpsimd.collective_compute(
            "AllGather", ALU.bypass,
            replica_groups=[list(range(n_cores))],
            ins=[cc2_in.opt()], outs=[cc2_out.opt()],
        )
        # prefetch the first gathered Vc0 blocks (fires as soon as cc2 lands);
        # the rest load from the scalar queue after phase A's last exp.
        for rc in range(n_cores // 2):
            nc.gpsimd.dma_start(
                vps[rc][:].bitcast(U8),
                cc2_out[rc, 0:VNB].rearrange("(p k) -> p k", p=P))

        # ------------------- phase A: scores / exp / CS+den -------------------
        with ExitStack() as pa:
            ktp = pa.enter_context(tc.tile_pool(name="kt", bufs=3))
            ztp = pa.enter_context(tc.tile_pool(name="zt", bufs=3, space="PSUM"))
            csp = pa.enter_context(tc.tile_pool(name="csp", bufs=1, space="PSUM"))
            sfp = pa.enter_context(tc.tile_pool(name="sfp", bufs=1, space="PSUM"))
            cs_ps = csp.tile([33, B], F32)

            q2T3 = q2T.rearrange("p (dc b) -> p dc b", dc=DC)
            pi = 0
            for rc in range(n_cores):
                ktc = ktp.tile([P, DC * B], FP8, tag="kt")
                nc.sync.dma_start(ktc[:], cc1_out[rc, 0:KH].rearrange("(p k) -> p k", p=P))
                ktc3 = ktc.rearrange("p (dc i) -> p dc i", dc=DC)
                for sub in range(TL):
                    rg = rc * TL + sub
                    zt = ztp.tile([P, B], F32, tag="zt")
                    for pp in range(DC // 2):
                        nc.tensor.matmul(
                            zt[:],
                            ktc3[:, 2 * pp:2 * pp + 2, sub * P:(sub + 1) * P],
                            q2T3[:, 2 * pp:2 * pp + 2, :],
                            start=(pp == 0), stop=(pp == DC // 2 - 1),
                            perf_mode=mybir.MatmulPerfMode.DoubleRow,
                        )
                    nc.scalar.activation(st[:, rg * B:(rg + 1) * B], zt[:],
                                         AF.Exp, bias=expb_sb[:], scale=scale)
                    if sub % 2 == 1:
                        pr = rg // 2
                        lp = (selp_sb[:, pr * 2 * MB:(pr + 1) * 2 * MB]
                              .rearrange("p (two m) -> p two m", two=2)[:, :, 0:33])
                        rp = (st[:, (rg - 1) * B:(rg + 1) * B]
                              .rearrange("p (two b) -> p two b", two=2))
                        nc.tensor.matmul(
                            cs_ps[:], lp, rp,
                            start=(pi == 0), stop=(pi == NPAIR - 1),
                            perf_mode=mybir.MatmulPerfMode.DoubleRow,
                        )
                        pi += 1

            for rc in range(n_cores // 2, n_cores):
                nc.scalar.dma_start(
                    vps[rc][:].bitcast(U8),
                    cc2_out[rc, 0:VNB].rearrange("(p k) -> p k", p=P))

            nc.vector.tensor_copy(cs_sb[:], cs_ps[0:T, :])
            nc.vector.memset(den_pad[:], 0.0)
            nc.vector.tensor_copy(den_pad[32:33, :], cs_ps[32:33, :])
            suf_ps = sfp.tile([T, B], F32)
            nc.tensor.matmul(suf_ps[:], stril_sb[:], cs_sb[:], start=True, stop=True)
            nc.vector.tensor_copy(suf_sb[:], suf_ps[:])

        # gathered x row sums -> VS = xrs.T-mm(wv)  [T, D]
        xg_u8 = xrs_g[:].bitcast(U8).rearrange("p (dc t2) -> p dc t2", dc=DC)
        for c in range(n_cores):
            nc.sync.dma_start(
                xg_u8[:, :, c * TL * 2:(c + 1) * TL * 2],
                cc2_out[c, VNB:VNB + XRB].rearrange("(p dc t2) -> p dc t2", p=P, dc=DC))
        with ExitStack() as svs:
            vsps = svs.enter_context(tc.tile_pool(name="vsps", bufs=2, space="PSUM"))
            for nh in range(NH):
                vs_ps = vsps.tile([T, W], F32, tag="vs")
                for dci in range(DC):
                    nc.tensor.matmul(
                        vs_ps[:],
                        xrs_g[:, dci * T:(dci + 1) * T],
                        wv_sb[:, dci * D + nh * W: dci * D + (nh + 1) * W],
                        start=(dci == 0), stop=(dci == DC - 1),
                    )
                nc.vector.tensor_copy(vs_sb[:, nh * W:(nh + 1) * W], vs_ps[:])

        # ------------------- phase B: numT accumulation -------------------
        H = DC // 2
        D2 = D // 2
        with ExitStack() as pb:
            trp2 = pb.enter_context(tc.tile_pool(name="trp2", bufs=4, space="PSUM"))
            outp = pb.enter_context(tc.tile_pool(name="outp", bufs=4))
            nump = pb.enter_context(tc.tile_pool(name="nump", bufs=H, space="PSUM"))
            nsbp = pb.enter_context(tc.tile_pool(name="nsb", bufs=H))

            # 1/den (overlaps the first group's matmuls)
            for sub in range(TL):
                dps = trp2.tile([P, P], F32, tag="tr2")
                nc.tensor.transpose(dps[:], den_pad[:, sub * P:(sub + 1) * P], ident_sb[:])
                nc.vector.tensor_copy(dennat[:, sub:sub + 1], dps[:, 32:33])
            nc.vector.reciprocal(recip[:], dennat[:])

            for g in range(2):
                nums = [nump.tile([P, B], F32, tag="num", name=f"num_ps{g}_{i}")
                        for i in range(H)]
                for rc in range(n_cores):
                    vp3 = vps[rc].rearrange("p (t d) -> p t d", t=TL)
                    for pr in range(TL // 2):
                        m3 = (st[:, (rc * TL + 2 * pr) * B:(rc * TL + 2 * pr + 2) * B]
                              .rearrange("p (two b) -> p two b", two=2))
                        for i in range(H):
                            nc.tensor.matmul(
                                nums[i][:],
                                vp3[:, 2 * pr:2 * pr + 2, g * D2 + i * P: g * D2 + (i + 1) * P],
                                m3,
                                start=(rc == 0 and pr == 0), stop=False,
                                perf_mode=mybir.MatmulPerfMode.DoubleRow,
                            )
                for i in range(H):
                    dc2 = g * H + i
                    nc.tensor.matmul(
                        nums[i][:], vs_sb[:, dc2 * P:(dc2 + 1) * P], suf_sb[:],
                        start=False, stop=True,
                    )
                # group epilogue: copy out of PSUM, transpose back, scale, store
                num_sb = []
                for i in range(H):
                    t_ = nsbp.tile([P, B], F32, tag="nsb", name=f"num_sb{g}_{i}")
                    nc.vector.tensor_copy(t_[:], nums[i][:])
                    num_sb.append(t_)
                for sub in range(TL):
                    ot = outp.tile([P, D // 2], F32, tag="ot")
                    for i in range(H):
                        tps = trp2.tile([P, P], F32, tag="tr2")
                        nc.tensor.transpose(tps[:], num_sb[i][:, sub * P:(sub + 1) * P], ident_sb[:])
                        nc.vector.tensor_scalar(
                            ot[:, i * P:(i + 1) * P], tps[:], recip[:, sub:sub + 1], 0.03125,
                            op0=ALU.mult, op1=ALU.mult,
                        )
                    nc.sync.dma_start(
                        out.ap()[sub * P:(sub + 1) * P, g * (D // 2):(g + 1) * (D // 2)],
                        ot[:],
                    )

    nc.compile()
    return nc


def make_in_maps(x_full, wq, wk, wv, n_cores=8):
    import ml_dtypes
    bf = lambda a: np.ascontiguousarray(a).astype(ml_dtypes.bfloat16)
    f8 = lambda a: np.ascontiguousarray(a).astype(ml_dtypes.float8_e4m3)
    SEQ, D = x_full.shape
    T = SEQ // P
    B = SEQ // n_cores
    consts = make_consts(T)
    wq8 = f8(wq * 8.0)
    wkt2 = f8(wk.T * 2.0)
    wvb = bf(wv)
    in_maps = []
    for c in range(n_cores):
        m = {"x": bf(x_full[c * B:(c + 1) * B]),
             "wq": wq8, "wk": wkt2, "wv": wvb}
        m.update(consts)
        in_maps.append(m)
    return in_maps


def algo_ref(x, wq, wk, wv):
    """Numpy float64 reference of the restructured math (for validation)."""
    x = x.astype(np.float64)
    q2 = (x @ wq.astype(np.float64)) @ wk.astype(np.float64).T
    s = np.exp(q2 @ x.T / np.sqrt(x.shape[1]))
    Vc = np.cumsum(x @ wv.astype(np.float64), axis=0)
    num = s @ Vc
    den = s @ (np.arange(x.shape[0]) + 1.0)
    return (num / den[:, None]).astype(np.float32)


# ----------------------------------------------------------------------------
# Harness entry point: full (unsharded) inputs -> full output.
# ----------------------------------------------------------------------------
SEQ, D_IN, N_CORES = 4096, 1024, 8
_built = {}


def _get_nc(SEQ_=SEQ, D_=D_IN, n_cores=N_CORES):
    key = (SEQ_, D_, n_cores)
    if key not in _built:
        _built[key] = build(SEQ=SEQ_, D=D_, n_cores=n_cores)
    return _built[key]


def run(x, wq, wk, wv, trace=False, **spmd_kwargs):
    from concourse.bass_utils import run_bass_kernel_spmd

    x = np.ascontiguousarray(np.asarray(x, dtype=np.float32))
    wq = np.ascontiguousarray(np.asarray(wq, dtype=np.float32))
    wk = np.ascontiguousarray(np.asarray(wk, dtype=np.float32))
    wv = np.ascontiguousarray(np.asarray(wv, dtype=np.float32))
    n_cores = N_CORES
    nc = _get_nc(x.shape[0], x.shape[1], n_cores)
    in_maps = make_in_maps(x, wq, wk, wv, n_cores=n_cores)
    res = run_bass_kernel_spmd(nc, in_maps, list(range(n_cores)),
                               trace=trace, **spmd_kwargs)
    out = np.concatenate([res.results[c]["out"] for c in range(n_cores)], axis=0)
    return out, res


def kernel(x, wq, wk, wv):
    out, _ = run(x, wq, wk, wv, trace=False)
    return out


# revision 20
# speedup vs baseline: 1.0546x; 1.0546x over previous
"""Bass/Tile TRN2 kernel for nn_CausalAttention (softmax + tril-matmul renorm).

Math restructuring (per core, row block of B = SEQ/n_cores rows):
    q = x @ wq ; k = x @ wk ; v = x @ wv
    z = q @ k.T / sqrt(D) ;  s = exp(z)              (softmax norm cancels below)
    masked[i,j] = sum_{t>=j} s[i,t]                  (suffix sum == s @ tril)
    out = (masked @ v) / rowsum(masked)

v2 identities (vs the tril/suffix formulation):
    masked @ v       == s @ cumsum(v)                 -> contract s with prefix-V
    rowsum(masked)   == s @ (t+1)                     -> one weight column
    z = q @ k.T      == ((x@wq) @ wk.T) @ x.T         -> gather RAW x, not K

so the AllGather input (x.T in fp8) is ready ~6us into the kernel instead of
after a full projection, and the O(S^2) tril matmuls + psum copies vanish.

Per-tile decomposition (tile r of 128 keys, T tiles):
    Vc[rP+j] = Vc0_r[j] + sum_{r'<r} VS[r']           (within-tile prefix + offsets)
    numT = sum_r Vc0_r.T-mm(st_r) + VS.T-mm(SUF)      SUF[r] = sum_{r'>r} CS[r']
    den[i] = sum_t (t+1) s[t,i]                       (selector-pair matmul w/ CS)

Layouts: everything transposed ([feature/key on partitions, query on free]).
Prefix-x trick: the x-transpose matmuls use rhs=[I | U] (U=upper-tri ones) so a
single pass yields both x.T (fp8, scores+gather) and prefix-x.T (bf16, V path);
xrs (tile row-sums of x) is the last U-column, read from PSUM in f32.

Collectives: cc1 = AllGather(x.T fp8) triggered right after the transposes;
cc2 = AllGather(Vc0 fp8 + xrs bf16) after the V projection. Both on the
otherwise-empty GpSimd queue so nothing delays the trigger.
"""
import numpy as np
from contextlib import ExitStack

import concourse.bass as bass
import concourse.tile as tile
from concourse import bacc, mybir

F32 = mybir.dt.float32
BF16 = mybir.dt.bfloat16
FP8 = mybir.dt.float8e4
U8 = mybir.dt.uint8
AX = mybir.AxisListType
AF = mybir.ActivationFunctionType
ALU = mybir.AluOpType

P = 128
MB = 48          # selector pair block half-width (padded for DoubleRow step%16)


def make_consts(T):
    iu = np.concatenate([np.eye(P, dtype=np.float32),
                         np.triu(np.ones((P, P), np.float32))], axis=1)
    stril = np.tri(T, T, -1, dtype=np.float32)  # [r', r] = 1 if r' > r
    # selector pair blocks (DoubleRow over tile pairs a=2pr, b=2pr+1):
    # ko0 col a / ko1 col b = 1 (per-tile colsum -> CS rows); col 32 = den
    # weight (t+1)/32, pinned to partition 32 for the later row extraction.
    NPAIR = T // 2
    selp = np.zeros((P, NPAIR * 2 * MB), np.float32)
    for pr in range(NPAIR):
        a, b = 2 * pr, 2 * pr + 1
        blk = pr * 2 * MB
        selp[:, blk + a] = 1.0
        selp[:, blk + 32] = (P * a + np.arange(P) + 1.0) / 32.0
        selp[:, blk + MB + b] = 1.0
        selp[:, blk + MB + 32] = (P * b + np.arange(P) + 1.0) / 32.0
    ident = np.eye(P, dtype=np.float32)
    import ml_dtypes
    bf = lambda a: a.astype(ml_dtypes.bfloat16)
    f8 = lambda a: a.astype(ml_dtypes.float8_e4m3)
    return dict(c_iu=bf(iu), c_stril=bf(stril), c_selp=f8(selp), c_ident=ident)


def build(SEQ=4096, D=1024, n_cores=8):
    T = SEQ // P           # global 128-key tiles
    TL = T // n_cores      # local tiles per core
    B = P * TL             # rows per core
    DC = D // P            # feature chunks
    W = min(512, D)        # moving free width for D-wide matmuls
    NH = D // W
    NPAIR = T // 2
    assert B <= 512 and T <= P and D % W == 0 and SEQ % (P * n_cores) == 0
    # wq prescaled x8, wk.T prescaled x2 -> z = 512 * z_true
    scale = float(1.0 / np.sqrt(D) / 16.0)
    EXPB = float(-np.log(16.0))   # st = s/16 keeps fp8e4 range safe

    nc = bacc.Bacc("TRN2", target_bir_lowering=False, debug=False, num_devices=n_cores)

    x = nc.dram_tensor("x", [B, D], BF16, kind="ExternalInput")
    wq_d = nc.dram_tensor("wq", [D, D], FP8, kind="ExternalInput")
    wkt_d = nc.dram_tensor("wk", [D, D], FP8, kind="ExternalInput")   # wk.T * 2
    wv_d = nc.dram_tensor("wv", [D, D], BF16, kind="ExternalInput")
    c_iu = nc.dram_tensor("c_iu", [P, 2 * P], BF16, kind="ExternalInput")
    c_stril = nc.dram_tensor("c_stril", [T, T], BF16, kind="ExternalInput")
    c_selp = nc.dram_tensor("c_selp", [P, NPAIR * 2 * MB], FP8, kind="ExternalInput")
    c_ident = nc.dram_tensor("c_ident", [P, P], F32, kind="ExternalInput")
    out = nc.dram_tensor("out", [B, D], F32, kind="ExternalOutput")

    KH = D * B             # cc1: xT8 region [P, DC*B] fp8, flat (p k)
    VNB = B * D            # cc2 bytes: Vc0 region [P, TL*D] fp8e4
    XRB = 2 * D * TL       # cc2 bytes: xrs region [P, DC*TL] bf16
    CC2N = VNB + XRB

    with tile.TileContext(nc) as tc, ExitStack() as top:
        dram = top.enter_context(tc.tile_pool(name="dram", bufs=1, space="DRAM"))
        cc1_in = dram.tile([KH], FP8)
        cc1_out = dram.tile([n_cores, KH], FP8, addr_space="Shared")
        cc2_in = dram.tile([CC2N], U8)
        cc2_out = dram.tile([n_cores, CC2N], U8, addr_space="Shared")

        consts = top.enter_context(tc.tile_pool(name="consts", bufs=1))
        iu_sb = consts.tile([P, 2 * P], BF16)
        nc.scalar.dma_start(iu_sb[:], c_iu.ap())
        stril_sb = consts.tile([T, T], BF16)
        nc.scalar.dma_start(stril_sb[:], c_stril.ap())
        selp_sb = consts.tile([P, NPAIR * 2 * MB], FP8)
        nc.scalar.dma_start(selp_sb[:], c_selp.ap())
        ident_sb = consts.tile([P, P], F32)
        nc.scalar.dma_start(ident_sb[:], c_ident.ap())
        expb_sb = consts.tile([P, 1], F32)
        nc.vector.memset(expb_sb[:], EXPB)

        persist = top.enter_context(tc.tile_pool(name="persist", bufs=1))
        q2T = persist.tile([P, DC * B], FP8)         # (q @ wk.T).T row block
        st = persist.tile([P, T * B], FP8)           # exp(scores)/16, transposed
        vs_sb = persist.tile([T, D], BF16)           # per-tile V colsums
        xrs_s = persist.tile([P, n_cores * DC * TL], BF16)  # gathered x row sums [p,(c dc t)]
        xrs_g = persist.tile([P, DC * T], BF16)             # re-strided to [p,(dc r)]
        suf_sb = persist.tile([T, B], BF16)
        cs_sb = persist.tile([T, B], BF16)
        recip = persist.tile([P, TL], F32)
        dennat = persist.tile([P, TL], F32)
        den_pad = persist.tile([P, B], F32)
        wv_sb = persist.tile([P, DC * D], BF16)
        vps = [persist.tile([P, TL * D], FP8, name=f"vp_{rc}")
               for rc in range(n_cores)]

        # ------------- stage 1: transposes, gather-x, projections -------------
        with ExitStack() as s1:
            xpool = s1.enter_context(tc.tile_pool(name="xload", bufs=6))
            xTp = s1.enter_context(tc.tile_pool(name="xT", bufs=1))
            xT8 = xTp.tile([P, DC * B], FP8)     # x.T   (scores lhsT + cc1 input)
            xcT = xTp.tile([P, DC * B], BF16)    # prefix-x.T (V path)
            xrs_f = xTp.tile([P, DC * TL], F32)
            xrs_bf = xTp.tile([P, DC * TL], BF16)

            wpool = s1.enter_context(tc.tile_pool(name="w", bufs=1))
            wq_sb = wpool.tile([P, DC * D], FP8)
            wkt_sb = wpool.tile([P, DC * D], FP8)
            qT = wpool.tile([P, DC * B], FP8)

            trps = s1.enter_context(tc.tile_pool(name="trps", bufs=2, space="PSUM"))
            # x.T first (I pass) -> cc1 trigger ASAP; prefix-x.T (U pass) after
            xts = []
            for tcc in range(TL):
                xt_ = xpool.tile([P, D], BF16, tag="x", name=f"xt_{tcc}")
                (nc.sync if tcc < 2 else nc.scalar).dma_start(
                    xt_[:], x.ap()[tcc * P:(tcc + 1) * P, :])
                xts.append(xt_)
            cc1v = cc1_in[0:KH].rearrange("(p k) -> p k", p=P)
            for dc in range(DC):
                psI = trps.tile([P, B], F32, tag="trI")
                for tcc in range(TL):
                    nc.tensor.matmul(psI[:, tcc * P:(tcc + 1) * P],
                                     xts[tcc][:, dc * P:(dc + 1) * P], iu_sb[:, 0:P],
                                     start=True, stop=True)
                (nc.vector.tensor_copy if dc % 2 == 0 else nc.scalar.copy)(
                    xT8[:, dc * B:(dc + 1) * B], psI[:])
                # stream x.T chunks to the collective input as they land
                nc.sync.dma_start(cc1v[:, dc * B:(dc + 1) * B],
                                  xT8[:, dc * B:(dc + 1) * B])

            # gather x.T immediately (GpSimd queue is otherwise empty)
            nc.gpsimd.collective_compute(
                "AllGather", ALU.bypass,
                replica_groups=[list(range(n_cores))],
                ins=[cc1_in.opt()], outs=[cc1_out.opt()],
            )

            for dc in range(DC):
                psU = trps.tile([P, B], F32, tag="trU")
                for tcc in range(TL):
                    nc.tensor.matmul(psU[:, tcc * P:(tcc + 1) * P],
                                     xts[tcc][:, dc * P:(dc + 1) * P], iu_sb[:, P:2 * P],
                                     start=True, stop=True)
                (nc.scalar.copy if dc % 2 == 0 else nc.vector.tensor_copy)(
                    xcT[:, dc * B:(dc + 1) * B], psU[:])
                nc.vector.tensor_copy(
                    xrs_f[:, dc * TL:(dc + 1) * TL]
                    .rearrange("p (t one) -> p t one", one=1),
                    psU.rearrange("p (t j) -> p t j", j=P)[:, :, P - 1:P])

            # weights (HWDGE queues; nothing gates the collective).
            # wv first: the V path gates cc2, which gates phase B.
            for dc in range(DC):
                nc.scalar.dma_start(wv_sb[:, dc * D:(dc + 1) * D], wv_d.ap()[dc * P:(dc + 1) * P, :])
            for dc in range(DC):
                nc.sync.dma_start(wq_sb[:, dc * D:(dc + 1) * D], wq_d.ap()[dc * P:(dc + 1) * P, :])
            for dc in range(DC):
                nc.scalar.dma_start(wkt_sb[:, dc * D:(dc + 1) * D], wkt_d.ap()[dc * P:(dc + 1) * P, :])

            pps = s1.enter_context(tc.tile_pool(name="pps", bufs=4, space="PSUM"))

            # Vc0 row blocks (within-tile prefix V, from prefix-x.T) -> cc2.
            # Emitted before the q projections: cc2 rides right behind cc1.
            cc2v = cc2_in[0:VNB].rearrange("(p k) -> p k", p=P)
            for tcc in range(TL):
                vl = xpool.tile([P, D], FP8, tag="vl")
                for nh in range(NH):
                    v_ps = pps.tile([P, W], F32, tag="pp", name="v_ps")
                    for dci in range(DC):
                        nc.tensor.matmul(
                            v_ps[:],
                            xcT[:, dci * B + tcc * P: dci * B + (tcc + 1) * P],
                            wv_sb[:, dci * D + nh * W: dci * D + (nh + 1) * W],
                            start=(dci == 0), stop=(dci == DC - 1),
                        )
                    (nc.vector.tensor_copy if nh % 2 == 0 else nc.scalar.copy)(
                        vl[:, nh * W:(nh + 1) * W], v_ps[:])
                nc.gpsimd.dma_start(cc2v[:, tcc * D:(tcc + 1) * D], vl[:].bitcast(U8))

            # per-tile x row sums (f32 exact, from the U-part last column)
            nc.vector.tensor_copy(xrs_bf[:], xrs_f[:])
            cc2x = cc2_in[VNB:VNB + XRB].rearrange("(p k) -> p k", p=P)
            nc.gpsimd.dma_start(cc2x[:, :], xrs_bf[:].bitcast(U8))

            # qT = (x @ wq).T  then  q2T = (q @ wk.T).T   (both fp8 DoubleRow)
            wq3 = wq_sb.rearrange("p (dc d) -> p dc d", dc=DC)
            wkt3 = wkt_sb.rearrange("p (dc d) -> p dc d", dc=DC)
            xT83 = xT8.rearrange("p (dc b) -> p dc b", dc=DC)
            for dco in range(DC):
                q_ps = pps.tile([P, B], F32, tag="pp", name="q_ps")
                for pp_ in range(DC // 2):
                    nc.tensor.matmul(
                        q_ps[:],
                        wq3[:, 2 * pp_:2 * pp_ + 2, dco * P:(dco + 1) * P],
                        xT83[:, 2 * pp_:2 * pp_ + 2, :],
                        start=(pp_ == 0), stop=(pp_ == DC // 2 - 1),
                        perf_mode=mybir.MatmulPerfMode.DoubleRow,
                    )
                nc.vector.tensor_copy(qT[:, dco * B:(dco + 1) * B], q_ps[:])
            qT3 = qT.rearrange("p (dc b) -> p dc b", dc=DC)
            for dco in range(DC):
                q2_ps = pps.tile([P, B], F32, tag="pp", name="q2_ps")
                for pp_ in range(DC // 2):
                    nc.tensor.matmul(
                        q2_ps[:],
                        wkt3[:, 2 * pp_:2 * pp_ + 2, dco * P:(dco + 1) * P],
                        qT3[:, 2 * pp_:2 * pp_ + 2, :],
                        start=(pp_ == 0), stop=(pp_ == DC // 2 - 1),
                        perf_mode=mybir.MatmulPerfMode.DoubleRow,
                    )
                nc.vector.tensor_copy(q2T[:, dco * B:(dco + 1) * B], q2_ps[:])

        # second collective: gather Vc0 + xrs
        nc.gpsimd.collective_compute(
            "AllGather", ALU.bypass,
            replica_groups=[list(range(n_cores))],
            ins=[cc2_in.opt()], outs=[cc2_out.opt()],
        )
        # prefetch the first gathered Vc0 blocks (fires as soon as cc2 lands)
        for rc in range(n_cores // 2):
            nc.gpsimd.dma_start(
                vps[rc][:].bitcast(U8),
                cc2_out[rc, 0:VNB].rearrange("(p k) -> p k", p=P))
        # gathered x row sums, per-core contiguous lines (128B per partition)
        for c in range(n_cores):
            nc.sync.dma_start(
                xrs_s[:, c * DC * TL:(c + 1) * DC * TL].bitcast(U8),
                cc2_out[c, VNB:VNB + XRB].rearrange("(p k) -> p k", p=P))

        # ------------------- phase A: scores / exp / CS+den -------------------
        with ExitStack() as pa:
            ktp = pa.enter_context(tc.tile_pool(name="kt", bufs=3))
            ztp = pa.enter_context(tc.tile_pool(name="zt", bufs=3, space="PSUM"))
            csp = pa.enter_context(tc.tile_pool(name="csp", bufs=1, space="PSUM"))
            sfp = pa.enter_context(tc.tile_pool(name="sfp", bufs=1, space="PSUM"))
            cs_ps = csp.tile([33, B], F32)

            q2T3 = q2T.rearrange("p (dc b) -> p dc b", dc=DC)
            pi = 0
            for rc in range(n_cores):
                ktc = ktp.tile([P, DC * B], FP8, tag="kt")
                nc.sync.dma_start(ktc[:], cc1_out[rc, 0:KH].rearrange("(p k) -> p k", p=P))
                ktc3 = ktc.rearrange("p (dc i) -> p dc i", dc=DC)
                for sub in range(TL):
                    rg = rc * TL + sub
                    zt = ztp.tile([P, B], F32, tag="zt")
                    for pp in range(DC // 2):
                        nc.tensor.matmul(
                            zt[:],
                            ktc3[:, 2 * pp:2 * pp + 2, sub * P:(sub + 1) * P],
                            q2T3[:, 2 * pp:2 * pp + 2, :],
                            start=(pp == 0), stop=(pp == DC // 2 - 1),
                            perf_mode=mybir.MatmulPerfMode.DoubleRow,
                        )
                    nc.scalar.activation(st[:, rg * B:(rg + 1) * B], zt[:],
                                         AF.Exp, bias=expb_sb[:], scale=scale)
                    if sub % 2 == 1:
                        pr = rg // 2
                        lp = (selp_sb[:, pr * 2 * MB:(pr + 1) * 2 * MB]
                              .rearrange("p (two m) -> p two m", two=2)[:, :, 0:33])
                        rp = (st[:, (rg - 1) * B:(rg + 1) * B]
                              .rearrange("p (two b) -> p two b", two=2))
                        nc.tensor.matmul(
                            cs_ps[:], lp, rp,
                            start=(pi == 0), stop=(pi == NPAIR - 1),
                            perf_mode=mybir.MatmulPerfMode.DoubleRow,
                        )
                        pi += 1

            for rc in range(n_cores // 2, n_cores):
                nc.sync.dma_start(
                    vps[rc][:].bitcast(U8),
                    cc2_out[rc, 0:VNB].rearrange("(p k) -> p k", p=P))

            nc.vector.tensor_copy(cs_sb[:], cs_ps[0:T, :])
            nc.vector.memset(den_pad[:], 0.0)
            nc.vector.tensor_copy(den_pad[32:33, :], cs_ps[32:33, :])
            suf_ps = sfp.tile([T, B], F32)
            nc.tensor.matmul(suf_ps[:], stril_sb[:], cs_sb[:], start=True, stop=True)
            nc.scalar.copy(suf_sb[:], suf_ps[:])

            # 0.03125/den now, so the phase-B epilogues are never gated on it
            for sub in range(TL):
                dps = sfp.tile([P, P], F32, tag="dtp")
                nc.tensor.transpose(dps[:], den_pad[:, sub * P:(sub + 1) * P], ident_sb[:])
                nc.vector.tensor_scalar(dennat[:, sub:sub + 1], dps[:, 32:33], 32.0,
                                        None, op0=ALU.mult)
            nc.vector.reciprocal(recip[:], dennat[:])

        # gathered x row sums -> VS = xrs.T-mm(wv)  [T, D]
        nc.vector.tensor_copy(
            xrs_g.rearrange("p (dc c t) -> p dc c t", dc=DC, c=n_cores),
            xrs_s.rearrange("p (c dc t) -> p dc c t", c=n_cores, dc=DC))
        with ExitStack() as svs:
            vsps = svs.enter_context(tc.tile_pool(name="vsps", bufs=2, space="PSUM"))
            for nh in range(NH):
                vs_ps = vsps.tile([T, W], F32, tag="vs")
                for dci in range(DC):
                    nc.tensor.matmul(
                        vs_ps[:],
                        xrs_g[:, dci * T:(dci + 1) * T],
                        wv_sb[:, dci * D + nh * W: dci * D + (nh + 1) * W],
                        start=(dci == 0), stop=(dci == DC - 1),
                    )
                nc.vector.tensor_copy(vs_sb[:, nh * W:(nh + 1) * W], vs_ps[:])

        # ------------------- phase B: numT accumulation -------------------
        H = DC // 2
        D2 = D // 2
        with ExitStack() as pb:
            trp2 = pb.enter_context(tc.tile_pool(name="trp2", bufs=4, space="PSUM"))
            outp = pb.enter_context(tc.tile_pool(name="outp", bufs=4))
            nump = pb.enter_context(tc.tile_pool(name="nump", bufs=H, space="PSUM"))
            nsbp = pb.enter_context(tc.tile_pool(name="nsb", bufs=H))

            for g in range(2):
                nums = [nump.tile([P, B], F32, tag="num", name=f"num_ps{g}_{i}")
                        for i in range(H)]
                for rc in range(n_cores):
                    vp3 = vps[rc].rearrange("p (t d) -> p t d", t=TL)
                    for pr in range(TL // 2):
                        m3 = (st[:, (rc * TL + 2 * pr) * B:(rc * TL + 2 * pr + 2) * B]
                              .rearrange("p (two b) -> p two b", two=2))
                        for i in range(H):
                            nc.tensor.matmul(
                                nums[i][:],
                                vp3[:, 2 * pr:2 * pr + 2, g * D2 + i * P: g * D2 + (i + 1) * P],
                                m3,
                                start=(rc == 0 and pr == 0), stop=False,
                                perf_mode=mybir.MatmulPerfMode.DoubleRow,
                            )
                # group epilogue: close + copy per chunk (two engines), then
                # per-sub transposes + scaled assembly, stores on both queues
                num_sb = []
                for i in range(H):
                    dc2 = g * H + i
                    nc.tensor.matmul(
                        nums[i][:], vs_sb[:, dc2 * P:(dc2 + 1) * P], suf_sb[:],
                        start=False, stop=True,
                    )
                    t_ = nsbp.tile([P, B], F32, tag="nsb", name=f"num_sb{g}_{i}")
                    (nc.vector.tensor_copy if i % 2 == 0 else nc.scalar.copy)(
                        t_[:], nums[i][:])
                    num_sb.append(t_)
                for sub in range(TL):
                    ot = outp.tile([P, D // 2], F32, tag="ot")
                    for i in range(H):
                        tps = trp2.tile([P, P], F32, tag="tr2")
                        nc.tensor.transpose(tps[:], num_sb[i][:, sub * P:(sub + 1) * P], ident_sb[:])
                        if i % 2 == 0:
                            nc.vector.tensor_scalar(
                                ot[:, i * P:(i + 1) * P], tps[:], recip[:, sub:sub + 1],
                                None, op0=ALU.mult)
                        else:
                            nc.scalar.activation(
                                ot[:, i * P:(i + 1) * P], tps[:], AF.Copy,
                                scale=recip[:, sub:sub + 1])
                    (nc.sync if sub % 2 == 0 else nc.scalar).dma_start(
                        out.ap()[sub * P:(sub + 1) * P, g * (D // 2):(g + 1) * (D // 2)],
                        ot[:],
                    )

    nc.compile()
    return nc


def make_in_maps(x_full, wq, wk, wv, n_cores=8):
    import ml_dtypes
    bf = lambda a: np.ascontiguousarray(a).astype(ml_dtypes.bfloat16)
    f8 = lambda a: np.ascontiguousarray(a).astype(ml_dtypes.float8_e4m3)
    SEQ, D = x_full.shape
    T = SEQ // P
    B = SEQ // n_cores
    consts = make_consts(T)
    wq8 = f8(wq * 8.0)
    wkt2 = f8(wk.T * 2.0)
    wvb = bf(wv)
    in_maps = []
    for c in range(n_cores):
        m = {"x": bf(x_full[c * B:(c + 1) * B]),
             "wq": wq8, "wk": wkt2, "wv": wvb}
        m.update(consts)
        in_maps.append(m)
    return in_maps


def algo_ref(x, wq, wk, wv):
    """Numpy float64 reference of the restructured math (for validation)."""
    x = x.astype(np.float64)
    q2 = (x @ wq.astype(np.float64)) @ wk.astype(np.float64).T
    s = np.exp(q2 @ x.T / np.sqrt(x.shape[1]))
    Vc = np.cumsum(x @ wv.astype(np.float64), axis=0)
    num = s @ Vc
    den = s @ (np.arange(x.shape[0]) + 1.0)
    return (num / den[:, None]).astype(np.float32)


# ----------------------------------------------------------------------------
# Harness entry point: full (unsharded) inputs -> full output.
# ----------------------------------------------------------------------------
SEQ, D_IN, N_CORES = 4096, 1024, 8
_built = {}


def _get_nc(SEQ_=SEQ, D_=D_IN, n_cores=N_CORES):
    key = (SEQ_, D_, n_cores)
    if key not in _built:
        _built[key] = build(SEQ=SEQ_, D=D_, n_cores=n_cores)
    return _built[key]


def run(x, wq, wk, wv, trace=False, **spmd_kwargs):
    from concourse.bass_utils import run_bass_kernel_spmd

    x = np.ascontiguousarray(np.asarray(x, dtype=np.float32))
    wq = np.ascontiguousarray(np.asarray(wq, dtype=np.float32))
    wk = np.ascontiguousarray(np.asarray(wk, dtype=np.float32))
    wv = np.ascontiguousarray(np.asarray(wv, dtype=np.float32))
    n_cores = N_CORES
    nc = _get_nc(x.shape[0], x.shape[1], n_cores)
    in_maps = make_in_maps(x, wq, wk, wv, n_cores=n_cores)
    res = run_bass_kernel_spmd(nc, in_maps, list(range(n_cores)),
                               trace=trace, **spmd_kwargs)
    out = np.concatenate([res.results[c]["out"] for c in range(n_cores)], axis=0)
    return out, res


def kernel(x, wq, wk, wv):
    out, _ = run(x, wq, wk, wv, trace=False)
    return out
